# revision 47
# baseline (speedup 1.0000x reference)
"""BiAttention kernel for Trainium2 (Bass/Tile), data-parallel over batch on 8 cores.

Math (per batch b):
  att[l,m] = s_in[l] + g[m] + S[l,m]
    S[l,m]  = sum_d inp[l,d]*dot_scale[d]*mem[m,d]
    s_in[l] = sum_d inp[l,d]*w_input[d]
    g[m]    = sum_d mem[m,d]*w_memory[d] + (mask[m]-1)*1e30
  weight_one = softmax_m(att) = softmax_m(S + g)           (s_in cancels)
  output_one = weight_one @ mem
  w2u[l] = exp(max_m att[l,:]) = max_m exp(S+g-2) * exp(s_in[l])   (shift cancels)
  output_two = (w2u/sum w2u) @ inp
  out = concat([inp, output_one, inp*output_one, output_two*output_one], -1)

v2: fp8 DoubleRow matmuls (2x PE) for both big matmuls.  S computed
transposed (S_T[m,l]); pt = exp(S_T + g - 2) written by ACT directly in
fp8 (the -2 shift keeps exp below the e4m3 max of 240; it cancels in both
softmaxes).  mm2 contracts pairs of m-tiles per instruction via DoubleRow
with an appended ones column giving the softmax denominators.  s_in and g
are computed on the PE as skinny DoubleRow matmuls against inT8/memT8
(w_input/dot_scale clamped to the fp8 range; g gets an fp8 residual
correction).  max_m tracked on fp8 pt, split DVE/GPSIMD by pair parity.
"""

import threading

import numpy as np

import concourse.bacc as bacc
import concourse.bass as bass
import concourse.mybir as mybir
import concourse.tile as tile
from concourse.masks import make_identity

F32 = mybir.dt.float32
F32R = mybir.dt.float32r
BF16 = mybir.dt.bfloat16
FP16 = mybir.dt.float16
FP8 = mybir.dt.float8e4
AF = mybir.ActivationFunctionType
ALU = mybir.AluOpType
AX = mybir.AxisListType
DR = mybir.MatmulPerfMode.DoubleRow

B, L, M, D = 8, 2048, 2048, 256
P = 128
LT = L // P          # 16 l-tiles
MT = M // P          # 16 m-tiles
KD = D // P          # 2 contraction tiles
NQ = 4               # l-quarters
QW = L // NQ         # 512
QT = QW // P         # 4 l-tiles per quarter
JP = MT // 2         # 8 m-tile pairs per quarter
NEG_BIG = 1.0e30
SHIFT = 2.0          # logit shift: keeps exp in fp8 range
W_CLAMP = 224.0      # fp8 e4m3 max normal is 240
# Pool has no fp8/uint8 ALU and no PSUM access: all max ops stay on DVE
MAX_ON_DVE = (True, True, True, True, True, True, True, True)


def build_nc():
    nc = bacc.Bacc(
        "TRN2", target_bir_lowering=False, debug=False, num_devices=8
    )

    inp_d = nc.dram_tensor("input", [L, D], F32, kind="ExternalInput").ap()
    mem_d = nc.dram_tensor("memory", [M, D], F32, kind="ExternalInput").ap()
    mask_d = nc.dram_tensor("mask", [M], F32, kind="ExternalInput").ap()
    w_in_d = nc.dram_tensor("w_input", [D], F32, kind="ExternalInput").ap()
    w_mem_d = nc.dram_tensor("w_memory", [D], F32, kind="ExternalInput").ap()
    dsc_d = nc.dram_tensor("dot_scale", [D], F32, kind="ExternalInput").ap()
    out_d = nc.dram_tensor("out", [L, 4 * D], F32, kind="ExternalOutput").ap()

    inp_r = inp_d.rearrange("(t p) d -> p t d", p=P)      # [128,16,256]
    mem_r = mem_d.rearrange("(t p) d -> p t d", p=P)      # [128,16,256]
    mask_r = mask_d.rearrange("(t p) -> t p", p=P)        # [16,128]
    out_r = out_d.rearrange("(t p) c -> p t c", p=P)      # [128,16,1024]

    with tile.TileContext(nc) as tc:
        with (
            tc.tile_pool(name="consts", bufs=1) as cp,
            tc.tile_pool(name="ptiles", bufs=4) as pp,
            tc.tile_pool(name="rp", bufs=4) as rp,
            tc.tile_pool(name="psS", bufs=1, space="PSUM") as psS,
            tc.tile_pool(name="psM", bufs=3, space="PSUM") as psM,
            tc.tile_pool(name="psAcc", bufs=4, space="PSUM") as psA,
        ):
            # ---------------- persistent SBUF ----------------
            ident_f = cp.tile([P, P], F32)
            ident_8 = cp.tile([P, P], FP8)
            ident_h = cp.tile([P, P], FP16)

            in_sb = cp.tile([P, LT, D], F32)        # natural input (f32)
            mem16 = cp.tile([P, MT, D + 1], FP16)   # natural memory fp16 + ones col
            inT8 = cp.tile([P, KD, L], FP8)         # fp8(inp*dsc)^T  [d,k,l]
            memT8 = cp.tile([P, KD, M], FP8)        # fp8(mem)^T      [d,k,m]
            maxD2 = cp.tile([P, NQ, 2, QW], FP16)   # running max acc (pair halves)
            out1_sb = cp.tile([P, LT, D], F32)      # normalized output_one
            st_sb = cp.tile([P, LT, D], F32)        # inp*out1 staging
            o4_sb = cp.tile([P, LT, D], F32)        # out2*out1 staging
            mask_pad = cp.tile([P, P], F32)         # mask rows 0:16
            vpad = cp.tile([P, P], F32)             # dsc rows 0:2, w_mem 2:4, w_in 4:6
            dsc_sb = cp.tile([P, KD], F32)
            rdsc = cp.tile([P, KD], F32)
            wq = cp.tile([P, KD], F32)
            w_in8 = cp.tile([P, KD], FP8)           # fp8(clamp(w_input/dsc))
            w_mem8a = cp.tile([P, KD], FP8)
            w_mem8af = cp.tile([P, KD], F32)
            w_mem8b = cp.tile([P, KD], FP8)
            g_sb = cp.tile([P, MT], F32)            # g + maskterm - SHIFT
            gtmp = cp.tile([P, MT], F32)
            s_in_sb = cp.tile([P, LT], F32)         # s_in dots (evacuated)
            o2acc = cp.tile([1, D], F32)            # output_two accumulator
            exp_si = cp.tile([P, LT], F32)          # exp(s_in)
            rowmax = cp.tile([P, LT], F32)
            w2u = cp.tile([P, LT], F32)
            w2u_r = cp.tile([P, LT], F32R)
            in_r = cp.tile([P, LT, D], F32R)        # f32r input for output_two
            w2s = cp.tile([P, 1], F32)
            cq = cp.tile([P, QW], FP16)             # per-quarter combined max
            ones_col = cp.tile([P, 1], F32)
            ones_row = cp.tile([1, P], F32)
            rtot = cp.tile([1, 1], F32)
            o2n = cp.tile([1, D], F32)
            o2b = cp.tile([P, D], F32)
            warm = cp.tile([P, 1], F32)

            # ---------------- loads ----------------
            # SP queue: input chunks first (chunk 0 gates the main loop)
            for c in range(8):
                nc.sync.dma_start(
                    out=in_sb[:, c * 2 : (c + 1) * 2, :],
                    in_=inp_r[:, c * 2 : (c + 1) * 2, :],
                )
            # ACT queue: small params + warm (table load off critical path)
            nc.scalar.dma_start(out=vpad[0:KD, :], in_=dsc_d.rearrange("(k p) -> k p", p=P))
            nc.scalar.dma_start(
                out=vpad[KD : 2 * KD, :], in_=w_mem_d.rearrange("(k p) -> k p", p=P)
            )
            nc.scalar.dma_start(
                out=vpad[2 * KD : 3 * KD, :], in_=w_in_d.rearrange("(k p) -> k p", p=P)
            )
            nc.scalar.dma_start(out=mask_pad[0:MT, :], in_=mask_r)
            # Pool queue: first two memory cast-loads lead (they gate the
            # first transposes), then the constants, then the rest
            for c in range(2):
                nc.gpsimd.dma_start(
                    out=mem16[:, 2 * c : 2 * c + 2, 0:D],
                    in_=mem_r[:, 2 * c : 2 * c + 2, :],
                )  # f32 -> fp16 cast DMA
            make_identity(nc, ident_f)
            make_identity(nc, ident_8)
            make_identity(nc, ident_h)
            nc.gpsimd.memset(ones_col[:], 1.0)
            nc.gpsimd.memset(ones_row[:], 1.0)
            nc.gpsimd.memset(mem16[:, :, D : D + 1], 1.0)
            nc.scalar.activation(out=warm[:], in_=ones_col[:], func=AF.Exp)

            # ---------------- small params ----------------
            pv = psS.tile([P, P], F32, tag="s", name="pv")
            nc.tensor.transpose(pv[:], vpad[:], ident_f[:])
            nc.vector.tensor_copy(dsc_sb[:], pv[:, 0:KD])
            nc.vector.reciprocal(rdsc[:], pv[:, 0:KD])
            # w' = clamp(w_input/dsc) in fp8 for the s_in dot
            nc.vector.tensor_mul(wq[:], pv[:, 2 * KD : 3 * KD], rdsc[:])
            nc.vector.tensor_scalar_min(out=wq[:], in0=wq[:], scalar1=W_CLAMP)
            nc.vector.tensor_scalar_max(out=wq[:], in0=wq[:], scalar1=-W_CLAMP)
            nc.vector.tensor_copy(w_in8[:], wq[:])
            # w_memory in fp8 + residual (g accuracy)
            nc.vector.tensor_copy(w_mem8a[:], pv[:, KD : 2 * KD])
            nc.vector.tensor_copy(w_mem8af[:], w_mem8a[:])
            nc.vector.tensor_sub(wq[:], pv[:, KD : 2 * KD], w_mem8af[:])
            nc.vector.tensor_copy(w_mem8b[:], wq[:])

            # mask term: (mask-1)*1e30 - SHIFT, in [m%128, mt] layout
            mtp = psS.tile([P, P], F32, tag="s", name="mtp")
            nc.tensor.transpose(mtp[:], mask_pad[:], ident_f[:])
            nc.vector.tensor_scalar(
                out=gtmp[:], in0=mtp[:, 0:MT], scalar1=1.0, scalar2=NEG_BIG,
                op0=ALU.subtract, op1=ALU.mult,
            )
            nc.vector.tensor_scalar(
                out=gtmp[:], in0=gtmp[:], scalar1=-SHIFT, scalar2=None, op0=ALU.add
            )

            # ---------------- transposes + evacuations ----------------
            def input_chunk(c):
                t0 = 2 * c
                ptr = psM.tile([P, 512], F32, tag="m", name=f"ti{c}")
                j = 0
                for k in range(KD):
                    for t in (t0, t0 + 1):
                        nc.tensor.transpose(
                            ptr[:, j * P : (j + 1) * P],
                            in_sb[:, t, k * P : (k + 1) * P],
                            ident_f,
                        )
                        j += 1
                ptf = ptr
                for k in range(KD):
                    nc.vector.tensor_scalar(
                        out=inT8[:, k, t0 * P : (t0 + 2) * P],
                        in0=ptf[:, k * 2 * P : (k + 1) * 2 * P],
                        scalar1=dsc_sb[:, k : k + 1], scalar2=None,
                        op0=ALU.mult,
                    )
                # s_in dot for the two l-tiles (fp8 DoubleRow, N=1)
                dc = psS.tile([P, 2], F32, tag="s", name=f"di{c}")
                for t in (t0, t0 + 1):
                    nc.tensor.matmul(
                        dc[:, t - t0 : t - t0 + 1],
                        lhsT=inT8[:, :, t * P : (t + 1) * P],
                        rhs=w_in8[:].unsqueeze(2),
                        start=True, stop=True, perf_mode=DR,
                    )
                nc.vector.tensor_copy(s_in_sb[:, t0 : t0 + 2], dc[:])

            def mem_chunk(c):
                t0 = 2 * c
                # memory arrives as fp16 (cast DMA); transpose fp16 via PE,
                # evacuate psum -> fp8 memT8 (mm1 DoubleRow operand)
                ptr = psM.tile([P, 512], FP16, tag="m", name=f"tm{c}")
                j = 0
                for k in range(KD):
                    for t in (t0, t0 + 1):
                        nc.tensor.transpose(
                            ptr[:, j * P : (j + 1) * P],
                            mem16[:, t, k * P : (k + 1) * P],
                            ident_h,
                        )
                        j += 1
                nc.vector.tensor_copy(
                    out=memT8[:, :, t0 * P : (t0 + 2) * P],
                    in_=ptr[:].rearrange("p (k x) -> p k x", k=KD),
                )
                # g dots (fp8 + residual), then assemble g_sb slice
                dg = psS.tile([P, 2], F32, tag="s", name=f"dg{c}")
                for t in (t0, t0 + 1):
                    nc.tensor.matmul(
                        dg[:, t - t0 : t - t0 + 1],
                        lhsT=memT8[:, :, t * P : (t + 1) * P],
                        rhs=w_mem8a[:].unsqueeze(2),
                        start=True, stop=False, perf_mode=DR,
                    )
                    nc.tensor.matmul(
                        dg[:, t - t0 : t - t0 + 1],
                        lhsT=memT8[:, :, t * P : (t + 1) * P],
                        rhs=w_mem8b[:].unsqueeze(2),
                        start=False, stop=True, perf_mode=DR,
                    )
                nc.vector.tensor_add(
                    g_sb[:, t0 : t0 + 2], dg[:], gtmp[:, t0 : t0 + 2]
                )

            # process the first two chunks up-front; the rest interleave with
            # the main loop (emitted inside the q=0 pair loop below)
            for c in range(2, 8):
                nc.gpsimd.dma_start(
                    out=mem16[:, 2 * c : 2 * c + 2, 0:D],
                    in_=mem_r[:, 2 * c : 2 * c + 2, :],
                )  # f32 -> fp16 cast DMA
            for c in range(2):
                mem_chunk(c)
                input_chunk(c)

            # ---------------- main loop ----------------
            def qtail(q):
                """rowmax + w2u + output_two partial for quarter q."""
                nc.vector.tensor_max(cq[:], maxD2[:, q, 0, :], maxD2[:, q, 1, :])
                rxp = psS.tile([P, QT, P], FP16, tag="s", name=f"rxp{q}")
                for lt in range(QT):
                    nc.tensor.transpose(
                        rxp[:, lt, :],
                        cq[:, lt * P : (lt + 1) * P],
                        ident_h,
                    )
                nc.vector.reduce_max(
                    rowmax[:, q * QT : (q + 1) * QT],
                    rxp[:],
                    axis=AX.X,
                )
                nc.vector.tensor_mul(
                    w2u[:, q * QT : (q + 1) * QT],
                    rowmax[:, q * QT : (q + 1) * QT],
                    exp_si[:, q * QT : (q + 1) * QT],
                )
                nc.vector.tensor_copy(
                    w2u_r[:, q * QT : (q + 1) * QT],
                    w2u[:, q * QT : (q + 1) * QT],
                )
                o2q = psS.tile([1, D], F32, tag="s", name=f"o2q{q}")
                for lt in range(QT):
                    tg = q * QT + lt
                    nc.tensor.matmul(
                        o2q[:],
                        lhsT=w2u_r[:, tg : tg + 1],
                        rhs=in_r[:, tg, :],
                        start=(lt == 0),
                        stop=(lt == QT - 1),
                    )
                if q == 0:
                    nc.vector.tensor_copy(o2acc[:], o2q[:])
                else:
                    nc.vector.tensor_add(o2acc[:], o2acc[:], o2q[:])

            def quarter_norm(q):
                """normalize output_one, stage blocks 1/2, write 0/1/2."""
                for lt in range(QT):
                    tg = q * QT + lt
                    acc = accs[lt]
                    r = rp.tile([P, 1], F32)
                    nc.vector.reciprocal(r[:], acc[:, D : D + 1])
                    if lt % 2 == 0:
                        # ACT does half the normalizes (copy with scale)
                        nc.scalar.activation(
                            out=out1_sb[:, tg, :], in_=acc[:, 0:D],
                            func=AF.Copy, scale=r[:],
                        )
                    else:
                        nc.vector.tensor_scalar(
                            out=out1_sb[:, tg, :], in0=acc[:, 0:D],
                            scalar1=r[:], scalar2=None, op0=ALU.mult,
                        )
                    nc.gpsimd.tensor_mul(
                        st_sb[:, tg, :], in_sb[:, tg, :], out1_sb[:, tg, :]
                    )
                qs = slice(q * QT, (q + 1) * QT)
                nc.sync.dma_start(out=out_r[:, qs, 0:D], in_=in_sb[:, qs, :])
                nc.sync.dma_start(out=out_r[:, qs, D : 2 * D], in_=out1_sb[:, qs, :])
                nc.sync.dma_start(out=out_r[:, qs, 2 * D : 3 * D], in_=st_sb[:, qs, :])

            # mm1 runs one iteration (m-tile) ahead of exp/mm2 so the PE and
            # ACT pipelines overlap; one psum tile per m-tile, psM bufs=2.
            def emit_mm1(q, t, ps):
                nc.tensor.matmul(
                    ps[:],
                    lhsT=memT8[:, :, t * P : (t + 1) * P],
                    rhs=inT8[:, :, q * QW : (q + 1) * QW],
                    start=True, stop=True, perf_mode=DR,
                )

            # two-iteration lookahead: exp(t) never waits on a fresh mm1
            psq = []
            for nt in range(2):
                pst = psM.tile([P, QW], F32, tag="m", name=f"ps_{nt}")
                emit_mm1(nt // MT, nt % MT, pst)
                psq.append(pst)
            pt = None
            for q in range(NQ):
                accs = [
                    psA.tile([P, D + 1], F32, tag="acc", name=f"acc_q{q}_{i}")
                    for i in range(QT)
                ]
                for t in range(MT):
                    j, i = t // 2, t % 2
                    ps = psq.pop(0)
                    nt = q * MT + t + 2
                    if nt < NQ * MT:
                        pst = psM.tile([P, QW], F32, tag="m", name=f"ps_{nt}")
                        emit_mm1(nt // MT, nt % MT, pst)
                        psq.append(pst)
                    if i == 0:
                        pt = pp.tile([P, 2, QW], FP16, name=f"pt_{q}_{j}", tag="pt")
                    nc.scalar.activation(
                        out=pt[:, i, :], in_=ps[:], func=AF.Exp,
                        bias=g_sb[:, t : t + 1],
                    )
                    if i == 1:
                        # running max over m (monotone exp): fp16 pair, 2x DVE
                        ptv = pt[:].rearrange("p i l -> p (i l)")
                        mv = maxD2[:, q, :, :].rearrange("p i l -> p (i l)")
                        if j == 0:
                            nc.vector.tensor_copy(mv, ptv)
                        else:
                            nc.vector.tensor_max(mv, mv, ptv)
                    # stream the remaining preamble chunks inside quarter 0
                    if q == 0 and i == 1 and 2 <= j + 2 < 8:
                        mem_chunk(j + 2)
                        input_chunk(j + 2)
                    if q == 0 and t == MT - 1:
                        nc.scalar.activation(
                            out=exp_si[:], in_=s_in_sb[:], func=AF.Exp
                        )
                    # f32r input copies for quarter q, needed at qtail(q)
                    if t in (8, 10):
                        ti = q * QT + (t - 8)
                        nc.vector.tensor_copy(
                            in_r[:, ti : ti + 2, :], in_sb[:, ti : ti + 2, :]
                        )
                    # mid-quarter: previous quarter's rowmax/out2
                    if t == 5 and q > 0:
                        qtail(q - 1)
                    if i == 1:
                        # output_one accumulation for the completed pair
                        for lt in range(QT):
                            for ii, tt in enumerate((t - 1, t)):
                                nc.tensor.matmul(
                                    accs[lt][:],
                                    lhsT=pt[:, ii, lt * P : (lt + 1) * P],
                                    rhs=mem16[:, tt, :],
                                    start=(t == 1 and ii == 0),
                                    stop=(t == MT - 1 and ii == 1),
                                )
                quarter_norm(q)
            qtail(NQ - 1)

            # ---------------- weight_two tail ----------------
            nc.vector.reduce_sum(w2s[:], w2u[:], axis=AX.X)
            totp = psM.tile([1, 1], F32, tag="m", name="totp")
            nc.tensor.matmul(totp[:], lhsT=w2s[:], rhs=ones_col[:], start=True, stop=True)
            nc.vector.reciprocal(rtot[:], totp[:])
            nc.vector.tensor_scalar(
                out=o2n[:], in0=o2acc[:], scalar1=rtot[:], scalar2=None, op0=ALU.mult
            )
            o2bp = psM.tile([P, D], F32, tag="m", name="o2bp")
            nc.tensor.matmul(
                o2bp[:], lhsT=ones_row[:], rhs=o2n[:], start=True, stop=True
            )
            nc.vector.tensor_copy(o2b[:], o2bp[:])

            for tg in range(LT):
                if tg % 2 == 0:
                    nc.vector.tensor_mul(o4_sb[:, tg, :], o2b[:], out1_sb[:, tg, :])
                else:
                    nc.gpsimd.tensor_mul(o4_sb[:, tg, :], o2b[:], out1_sb[:, tg, :])
                if tg % 4 == 3:
                    sl = slice(tg - 3, tg + 1)
                    if tg % 8 == 3:
                        nc.sync.dma_start(
                            out=out_r[:, sl, 3 * D : 4 * D], in_=o4_sb[:, sl, :]
                        )
                    else:
                        nc.gpsimd.dma_start(
                            out=out_r[:, sl, 3 * D : 4 * D], in_=o4_sb[:, sl, :]
                        )

    nc.compile()
    return nc


_CACHE = threading.local()


def _get_nc():
    nc = getattr(_CACHE, "nc", None)
    if nc is None:
        nc = build_nc()
        _CACHE.nc = nc
    return nc


def make_in_maps(input, memory, mask, w_input, w_memory, dot_scale):
    input = np.ascontiguousarray(np.asarray(input, dtype=np.float32))
    memory = np.ascontiguousarray(np.asarray(memory, dtype=np.float32))
    mask = np.ascontiguousarray(np.asarray(mask, dtype=np.float32))
    w_input = np.ascontiguousarray(np.asarray(w_input, dtype=np.float32))
    w_memory = np.ascontiguousarray(np.asarray(w_memory, dtype=np.float32))
    dot_scale = np.ascontiguousarray(np.asarray(dot_scale, dtype=np.float32))
    return [
        {
            "input": input[b],
            "memory": memory[b],
            "mask": mask[b],
            "w_input": w_input,
            "w_memory": w_memory,
            "dot_scale": dot_scale,
        }
        for b in range(B)
    ]


def _run_once(nc, in_maps):
    from concourse.bass_utils import run_bass_kernel_spmd

    res = run_bass_kernel_spmd(nc, in_maps, core_ids=list(range(B)))
    return np.stack([res.results[b]["out"] for b in range(B)], axis=0)


def kernel(input, memory, mask, w_input, w_memory, dot_scale):
    nc = _get_nc()
    in_maps = make_in_maps(input, memory, mask, w_input, w_memory, dot_scale)
    # The kernel is deterministic; rarely a core returns corrupted data after
    # an earlier device fault.  Run twice and require agreement.
    out = _run_once(nc, in_maps)
    for _ in range(3):
        out2 = _run_once(nc, in_maps)
        if np.array_equal(out, out2):
            return out
        out = out2
    return out


# revision 53
# speedup vs baseline: 1.1382x; 1.1382x over previous
"""BiAttention kernel for Trainium2 (Bass/Tile), data-parallel over batch on 8 cores.

Math (per batch b):
  att[l,m] = s_in[l] + g[m] + S[l,m]
    S[l,m]  = sum_d inp[l,d]*dot_scale[d]*mem[m,d]
    s_in[l] = sum_d inp[l,d]*w_input[d]
    g[m]    = sum_d mem[m,d]*w_memory[d] + (mask[m]-1)*1e30
  weight_one = softmax_m(att) = softmax_m(S + g)           (s_in cancels)
  output_one = weight_one @ mem
  w2u[l] = exp(max_m att[l,:]) = max_m exp(S+g-2) * exp(s_in[l])   (shift cancels)
  output_two = (w2u/sum w2u) @ inp
  out = concat([inp, output_one, inp*output_one, output_two*output_one], -1)

v2: fp8 DoubleRow matmuls (2x PE) for both big matmuls.  S computed
transposed (S_T[m,l]); pt = exp(S_T + g - 2) written by ACT directly in
fp8 (the -2 shift keeps exp below the e4m3 max of 240; it cancels in both
softmaxes).  mm2 contracts pairs of m-tiles per instruction via DoubleRow
with an appended ones column giving the softmax denominators.  s_in and g
are computed on the PE as skinny DoubleRow matmuls against inT8/memT8
(w_input/dot_scale clamped to the fp8 range; g gets an fp8 residual
correction).  max_m tracked on fp8 pt, split DVE/GPSIMD by pair parity.
"""

import threading

import numpy as np

import concourse.bacc as bacc
import concourse.bass as bass
import concourse.mybir as mybir
import concourse.tile as tile
from concourse.masks import make_identity

F32 = mybir.dt.float32
F32R = mybir.dt.float32r
BF16 = mybir.dt.bfloat16
FP16 = mybir.dt.float16
FP8 = mybir.dt.float8e4
AF = mybir.ActivationFunctionType
ALU = mybir.AluOpType
AX = mybir.AxisListType
DR = mybir.MatmulPerfMode.DoubleRow

B, L, M, D = 8, 2048, 2048, 256
P = 128
LT = L // P          # 16 l-tiles
MT = M // P          # 16 m-tiles
KD = D // P          # 2 contraction tiles
NQ = 4               # l-quarters
QW = L // NQ         # 512
QT = QW // P         # 4 l-tiles per quarter
JP = MT // 2         # 8 m-tile pairs per quarter
NEG_BIG = 1.0e30
SHIFT = 2.0          # logit shift: keeps exp in fp8 range
W_CLAMP = 224.0      # fp8 e4m3 max normal is 240
# Pool has no fp8/uint8 ALU and no PSUM access: all max ops stay on DVE
MAX_ON_DVE = (True, True, True, True, True, True, True, True)


def build_nc():
    nc = bacc.Bacc(
        "TRN2", target_bir_lowering=False, debug=False, num_devices=8
    )

    inp_d = nc.dram_tensor("input", [L, D], F32, kind="ExternalInput").ap()
    mem_d = nc.dram_tensor("memory", [M, D], F32, kind="ExternalInput").ap()
    mask_d = nc.dram_tensor("mask", [M], F32, kind="ExternalInput").ap()
    w_in_d = nc.dram_tensor("w_input", [D], F32, kind="ExternalInput").ap()
    w_mem_d = nc.dram_tensor("w_memory", [D], F32, kind="ExternalInput").ap()
    dsc_d = nc.dram_tensor("dot_scale", [D], F32, kind="ExternalInput").ap()
    out_d = nc.dram_tensor("out", [L, 4 * D], F32, kind="ExternalOutput").ap()

    inp_r = inp_d.rearrange("(t p) d -> p t d", p=P)      # [128,16,256]
    mem_r = mem_d.rearrange("(t p) d -> p t d", p=P)      # [128,16,256]
    mask_r = mask_d.rearrange("(t p) -> t p", p=P)        # [16,128]
    out_r = out_d.rearrange("(t p) c -> p t c", p=P)      # [128,16,1024]

    with tile.TileContext(nc) as tc:
        with (
            tc.tile_pool(name="consts", bufs=1) as cp,
            tc.tile_pool(name="ptiles", bufs=4) as pp,
            tc.tile_pool(name="rp", bufs=4) as rp,
            tc.tile_pool(name="psS", bufs=1, space="PSUM") as psS,
            tc.tile_pool(name="psM", bufs=3, space="PSUM") as psM,
            tc.tile_pool(name="psAcc", bufs=4, space="PSUM") as psA,
        ):
            # ---------------- persistent SBUF ----------------
            ident_f = cp.tile([P, P], F32)
            ident_8 = cp.tile([P, P], FP8)
            ident_h = cp.tile([P, P], FP16)

            in_sb = cp.tile([P, LT, D], F32)        # natural input (f32)
            mem16 = cp.tile([P, MT, D + 1], FP16)   # natural memory fp16 + ones col
            inT8 = cp.tile([P, KD, L], FP8)         # fp8(inp*dsc)^T  [d,k,l]
            memT8 = cp.tile([P, KD, M], FP8)        # fp8(mem)^T      [d,k,m]
            maxD2 = cp.tile([P, NQ, 2, QW], FP16)   # running max acc (pair halves)
            out1_sb = cp.tile([P, LT, D], F32)      # normalized output_one
            st_sb = cp.tile([P, LT, D], F32)        # inp*out1 staging
            o4_sb = cp.tile([P, LT, D], F32)        # out2*out1 staging
            mask_pad = cp.tile([P, P], F32)         # mask rows 0:16
            vpad = cp.tile([P, P], F32)             # dsc rows 0:2, w_mem 2:4, w_in 4:6
            dsc_sb = cp.tile([P, KD], F32)
            rdsc = cp.tile([P, KD], F32)
            wq = cp.tile([P, KD], F32)
            w_in8 = cp.tile([P, KD], FP8)           # fp8(clamp(w_input/dsc))
            w_mem8a = cp.tile([P, KD], FP8)
            w_mem8af = cp.tile([P, KD], F32)
            w_mem8b = cp.tile([P, KD], FP8)
            g_sb = cp.tile([P, MT], F32)            # g + maskterm - SHIFT
            gtmp = cp.tile([P, MT], F32)
            s_in_sb = cp.tile([P, LT], F32)         # s_in dots (evacuated)
            o2acc = cp.tile([1, D], F32)            # output_two accumulator
            exp_si = cp.tile([P, LT], F32)          # exp(s_in)
            rowmax = cp.tile([P, LT], F32)
            w2u = cp.tile([P, LT], F32)
            w2u_r = cp.tile([P, LT], F32R)
            in_r = cp.tile([P, LT, D], F32R)        # f32r input for output_two
            w2s = cp.tile([P, 1], F32)
            cq = cp.tile([P, QW], FP16)             # per-quarter combined max
            ones_col = cp.tile([P, 1], F32)
            ones_row = cp.tile([1, P], F32)
            rtot = cp.tile([1, 1], F32)
            o2n = cp.tile([1, D], F32)
            o2b = cp.tile([P, D], F32)
            warm = cp.tile([P, 1], F32)

            # ---------------- loads ----------------
            # SP queue: input chunks first (chunk 0 gates the main loop)
            for c in range(8):
                nc.sync.dma_start(
                    out=in_sb[:, c * 2 : (c + 1) * 2, :],
                    in_=inp_r[:, c * 2 : (c + 1) * 2, :],
                )
            # ACT queue: small params + warm (table load off critical path)
            nc.scalar.dma_start(out=vpad[0:KD, :], in_=dsc_d.rearrange("(k p) -> k p", p=P))
            nc.scalar.dma_start(
                out=vpad[KD : 2 * KD, :], in_=w_mem_d.rearrange("(k p) -> k p", p=P)
            )
            nc.scalar.dma_start(
                out=vpad[2 * KD : 3 * KD, :], in_=w_in_d.rearrange("(k p) -> k p", p=P)
            )
            nc.scalar.dma_start(out=mask_pad[0:MT, :], in_=mask_r)
            # Pool queue: first two memory cast-loads lead (they gate the
            # first transposes), then the constants, then the rest
            for c in range(2):
                nc.gpsimd.dma_start(
                    out=mem16[:, 2 * c : 2 * c + 2, 0:D],
                    in_=mem_r[:, 2 * c : 2 * c + 2, :],
                )  # f32 -> fp16 cast DMA
            nc.gpsimd.memset(ones_col[:], 1.0)
            nc.scalar.activation(out=warm[:], in_=ones_col[:], func=AF.Exp)
            make_identity(nc, ident_f)
            make_identity(nc, ident_8)
            make_identity(nc, ident_h)
            nc.gpsimd.memset(ones_row[:], 1.0)
            nc.gpsimd.memset(mem16[:, :, D : D + 1], 1.0)

            # ---------------- small params ----------------
            pv = psS.tile([P, P], F32, tag="s", name="pv")
            nc.tensor.transpose(pv[:], vpad[:], ident_f[:])
            nc.vector.tensor_copy(dsc_sb[:], pv[:, 0:KD])
            nc.vector.reciprocal(rdsc[:], pv[:, 0:KD])
            # w' = clamp(w_input/dsc) in fp8 for the s_in dot
            nc.vector.tensor_mul(wq[:], pv[:, 2 * KD : 3 * KD], rdsc[:])
            nc.vector.tensor_scalar_min(out=wq[:], in0=wq[:], scalar1=W_CLAMP)
            nc.vector.tensor_scalar_max(out=wq[:], in0=wq[:], scalar1=-W_CLAMP)
            nc.vector.tensor_copy(w_in8[:], wq[:])
            # w_memory in fp8 + residual (g accuracy)
            nc.vector.tensor_copy(w_mem8a[:], pv[:, KD : 2 * KD])
            nc.vector.tensor_copy(w_mem8af[:], w_mem8a[:])
            nc.vector.tensor_sub(wq[:], pv[:, KD : 2 * KD], w_mem8af[:])
            nc.vector.tensor_copy(w_mem8b[:], wq[:])

            # mask term: (mask-1)*1e30 - SHIFT, in [m%128, mt] layout
            mtp = psS.tile([P, P], F32, tag="s", name="mtp")
            nc.tensor.transpose(mtp[:], mask_pad[:], ident_f[:])
            nc.vector.tensor_scalar(
                out=gtmp[:], in0=mtp[:, 0:MT], scalar1=1.0, scalar2=NEG_BIG,
                op0=ALU.subtract, op1=ALU.mult,
            )
            nc.vector.tensor_scalar(
                out=gtmp[:], in0=gtmp[:], scalar1=-SHIFT, scalar2=None, op0=ALU.add
            )

            # ---------------- transposes + evacuations ----------------
            def input_chunk(c):
                t0 = 2 * c
                ptr = psM.tile([P, 512], F32, tag="m", name=f"ti{c}")
                j = 0
                for k in range(KD):
                    for t in (t0, t0 + 1):
                        nc.tensor.transpose(
                            ptr[:, j * P : (j + 1) * P],
                            in_sb[:, t, k * P : (k + 1) * P],
                            ident_f,
                        )
                        j += 1
                ptf = ptr
                for k in range(KD):
                    nc.vector.tensor_scalar(
                        out=inT8[:, k, t0 * P : (t0 + 2) * P],
                        in0=ptf[:, k * 2 * P : (k + 1) * 2 * P],
                        scalar1=dsc_sb[:, k : k + 1], scalar2=None,
                        op0=ALU.mult,
                    )
                # s_in dot for the two l-tiles (fp8 DoubleRow, N=1)
                dc = psS.tile([P, 2], F32, tag="s", name=f"di{c}")
                for t in (t0, t0 + 1):
                    nc.tensor.matmul(
                        dc[:, t - t0 : t - t0 + 1],
                        lhsT=inT8[:, :, t * P : (t + 1) * P],
                        rhs=w_in8[:].unsqueeze(2),
                        start=True, stop=True, perf_mode=DR,
                    )
                nc.vector.tensor_copy(s_in_sb[:, t0 : t0 + 2], dc[:])

            def mem_chunk(c):
                t0 = 2 * c
                # memory arrives as fp16 (cast DMA); transpose fp16 via PE,
                # evacuate psum -> fp8 memT8 (mm1 DoubleRow operand)
                ptr = psM.tile([P, 512], FP16, tag="m", name=f"tm{c}")
                j = 0
                for k in range(KD):
                    for t in (t0, t0 + 1):
                        nc.tensor.transpose(
                            ptr[:, j * P : (j + 1) * P],
                            mem16[:, t, k * P : (k + 1) * P],
                            ident_h,
                        )
                        j += 1
                nc.vector.tensor_copy(
                    out=memT8[:, :, t0 * P : (t0 + 2) * P],
                    in_=ptr[:].rearrange("p (k x) -> p k x", k=KD),
                )
                # g dots (fp8 + residual), then assemble g_sb slice
                dg = psS.tile([P, 2], F32, tag="s", name=f"dg{c}")
                for t in (t0, t0 + 1):
                    nc.tensor.matmul(
                        dg[:, t - t0 : t - t0 + 1],
                        lhsT=memT8[:, :, t * P : (t + 1) * P],
                        rhs=w_mem8a[:].unsqueeze(2),
                        start=True, stop=False, perf_mode=DR,
                    )
                    nc.tensor.matmul(
                        dg[:, t - t0 : t - t0 + 1],
                        lhsT=memT8[:, :, t * P : (t + 1) * P],
                        rhs=w_mem8b[:].unsqueeze(2),
                        start=False, stop=True, perf_mode=DR,
                    )
                nc.vector.tensor_add(
                    g_sb[:, t0 : t0 + 2], dg[:], gtmp[:, t0 : t0 + 2]
                )

            # process the first two chunks up-front; the rest interleave with
            # the main loop (emitted inside the q=0 pair loop below)
            for c in range(2, 8):
                nc.gpsimd.dma_start(
                    out=mem16[:, 2 * c : 2 * c + 2, 0:D],
                    in_=mem_r[:, 2 * c : 2 * c + 2, :],
                )  # f32 -> fp16 cast DMA
            for c in range(2):
                input_chunk(c)
                mem_chunk(c)

            # ---------------- main loop ----------------
            def qtail(q):
                """rowmax + w2u + output_two partial for quarter q."""
                nc.vector.tensor_max(cq[:], maxD2[:, q, 0, :], maxD2[:, q, 1, :])
                rxp = psS.tile([P, QT, P], FP16, tag="s", name=f"rxp{q}")
                for lt in range(QT):
                    nc.tensor.transpose(
                        rxp[:, lt, :],
                        cq[:, lt * P : (lt + 1) * P],
                        ident_h,
                    )
                nc.vector.reduce_max(
                    rowmax[:, q * QT : (q + 1) * QT],
                    rxp[:],
                    axis=AX.X,
                )
                nc.vector.tensor_mul(
                    w2u[:, q * QT : (q + 1) * QT],
                    rowmax[:, q * QT : (q + 1) * QT],
                    exp_si[:, q * QT : (q + 1) * QT],
                )
                nc.vector.tensor_copy(
                    w2u_r[:, q * QT : (q + 1) * QT],
                    w2u[:, q * QT : (q + 1) * QT],
                )
                o2q = psS.tile([1, D], F32, tag="s", name=f"o2q{q}")
                for lt in range(QT):
                    tg = q * QT + lt
                    nc.tensor.matmul(
                        o2q[:],
                        lhsT=w2u_r[:, tg : tg + 1],
                        rhs=in_r[:, tg, :],
                        start=(lt == 0),
                        stop=(lt == QT - 1),
                    )
                if q == 0:
                    nc.vector.tensor_copy(o2acc[:], o2q[:])
                else:
                    nc.vector.tensor_add(o2acc[:], o2acc[:], o2q[:])

            def quarter_norm(q, stage=True):
                """normalize output_one; optionally stage blocks 0/1/2."""
                for lt in range(QT):
                    tg = q * QT + lt
                    acc = accs[lt]
                    r = rp.tile([P, 1], F32)
                    nc.vector.reciprocal(r[:], acc[:, D : D + 1])
                    if lt % 2 == 0:
                        # ACT does half the normalizes (copy with scale)
                        nc.scalar.activation(
                            out=out1_sb[:, tg, :], in_=acc[:, 0:D],
                            func=AF.Copy, scale=r[:],
                        )
                    else:
                        nc.vector.tensor_scalar(
                            out=out1_sb[:, tg, :], in0=acc[:, 0:D],
                            scalar1=r[:], scalar2=None, op0=ALU.mult,
                        )
                    if stage:
                        nc.gpsimd.tensor_mul(
                            st_sb[:, tg, :], in_sb[:, tg, :], out1_sb[:, tg, :]
                        )
                if stage:
                    quarter_stage(q)

            def quarter_stage(q):
                qs = slice(q * QT, (q + 1) * QT)
                nc.sync.dma_start(out=out_r[:, qs, 0:D], in_=in_sb[:, qs, :])
                nc.sync.dma_start(out=out_r[:, qs, D : 2 * D], in_=out1_sb[:, qs, :])
                nc.sync.dma_start(out=out_r[:, qs, 2 * D : 3 * D], in_=st_sb[:, qs, :])

            # mm1 runs one iteration (m-tile) ahead of exp/mm2 so the PE and
            # ACT pipelines overlap; one psum tile per m-tile, psM bufs=2.
            def emit_mm1(q, t, ps):
                nc.tensor.matmul(
                    ps[:],
                    lhsT=memT8[:, :, t * P : (t + 1) * P],
                    rhs=inT8[:, :, q * QW : (q + 1) * QW],
                    start=True, stop=True, perf_mode=DR,
                )

            # two-iteration lookahead: exp(t) never waits on a fresh mm1
            psq = []
            for nt in range(2):
                pst = psM.tile([P, QW], F32, tag="m", name=f"ps_{nt}")
                emit_mm1(nt // MT, nt % MT, pst)
                psq.append(pst)
            pt = None
            for q in range(NQ):
                accs = [
                    psA.tile([P, D + 1], F32, tag="acc", name=f"acc_q{q}_{i}")
                    for i in range(QT)
                ]
                for t in range(MT):
                    j, i = t // 2, t % 2
                    ps = psq.pop(0)
                    nt = q * MT + t + 2
                    if nt < NQ * MT:
                        pst = psM.tile([P, QW], F32, tag="m", name=f"ps_{nt}")
                        emit_mm1(nt // MT, nt % MT, pst)
                        psq.append(pst)
                    if i == 0:
                        pt = pp.tile([P, 2, QW], FP16, name=f"pt_{q}_{j}", tag="pt")
                    nc.scalar.activation(
                        out=pt[:, i, :], in_=ps[:], func=AF.Exp,
                        bias=g_sb[:, t : t + 1],
                    )
                    if i == 1:
                        # running max over m (monotone exp): fp16 pair, 2x DVE
                        ptv = pt[:].rearrange("p i l -> p (i l)")
                        mv = maxD2[:, q, :, :].rearrange("p i l -> p (i l)")
                        if j == 0:
                            nc.vector.tensor_copy(mv, ptv)
                        else:
                            nc.vector.tensor_max(mv, mv, ptv)
                    # stream the remaining preamble chunks inside quarter 0
                    if q == 0 and i == 1 and 2 <= j + 2 < 8:
                        input_chunk(j + 2)
                        mem_chunk(j + 2)
                    if q == 0 and t == MT - 1:
                        nc.scalar.activation(
                            out=exp_si[:], in_=s_in_sb[:], func=AF.Exp
                        )
                    # f32r input copies for quarter q, needed at qtail(q)
                    if t in (8, 10):
                        ti = q * QT + (t - 8)
                        nc.vector.tensor_copy(
                            in_r[:, ti : ti + 2, :], in_sb[:, ti : ti + 2, :]
                        )
                    # mid-quarter: previous quarter's rowmax/out2
                    if t == 5 and q > 0:
                        qtail(q - 1)
                    if i == 1:
                        # output_one accumulation for the completed pair
                        for lt in range(QT):
                            for ii, tt in enumerate((t - 1, t)):
                                nc.tensor.matmul(
                                    accs[lt][:],
                                    lhsT=pt[:, ii, lt * P : (lt + 1) * P],
                                    rhs=mem16[:, tt, :],
                                    start=(t == 1 and ii == 0),
                                    stop=(t == MT - 1 and ii == 1),
                                )
                # q3's block-2 staging is off the critical path: defer it
                # until after the output_two tail so o4 starts immediately
                quarter_norm(q, stage=(q < NQ - 1))
            qtail(NQ - 1)

            # ---------------- weight_two tail ----------------
            nc.vector.reduce_sum(w2s[:], w2u[:], axis=AX.X)
            totp = psM.tile([1, 1], F32, tag="m", name="totp")
            nc.tensor.matmul(totp[:], lhsT=w2s[:], rhs=ones_col[:], start=True, stop=True)
            nc.vector.reciprocal(rtot[:], totp[:])
            nc.vector.tensor_scalar(
                out=o2n[:], in0=o2acc[:], scalar1=rtot[:], scalar2=None, op0=ALU.mult
            )
            o2bp = psM.tile([P, D], F32, tag="m", name="o2bp")
            nc.tensor.matmul(
                o2bp[:], lhsT=ones_row[:], rhs=o2n[:], start=True, stop=True
            )
            nc.vector.tensor_copy(o2b[:], o2bp[:])

            for tg in range(LT):
                # DVE is faster per-op: give it 10 of 16 tiles
                if tg % 8 in (0, 2, 4, 5, 6):
                    nc.vector.tensor_mul(o4_sb[:, tg, :], o2b[:], out1_sb[:, tg, :])
                else:
                    nc.gpsimd.tensor_mul(o4_sb[:, tg, :], o2b[:], out1_sb[:, tg, :])
                if tg % 2 == 1:
                    sl = slice(tg - 1, tg + 1)
                    if tg % 4 == 1:
                        nc.sync.dma_start(
                            out=out_r[:, sl, 3 * D : 4 * D], in_=o4_sb[:, sl, :]
                        )
                    else:
                        nc.gpsimd.dma_start(
                            out=out_r[:, sl, 3 * D : 4 * D], in_=o4_sb[:, sl, :]
                        )
            # deferred q3 block-2 staging (after the latency-critical o4s)
            for lt in range(QT):
                tg = (NQ - 1) * QT + lt
                nc.gpsimd.tensor_mul(
                    st_sb[:, tg, :], in_sb[:, tg, :], out1_sb[:, tg, :]
                )
            quarter_stage(NQ - 1)

    nc.compile()
    return nc


_CACHE = threading.local()


def _get_nc():
    nc = getattr(_CACHE, "nc", None)
    if nc is None:
        nc = build_nc()
        _CACHE.nc = nc
    return nc


def make_in_maps(input, memory, mask, w_input, w_memory, dot_scale):
    input = np.ascontiguousarray(np.asarray(input, dtype=np.float32))
    memory = np.ascontiguousarray(np.asarray(memory, dtype=np.float32))
    mask = np.ascontiguousarray(np.asarray(mask, dtype=np.float32))
    w_input = np.ascontiguousarray(np.asarray(w_input, dtype=np.float32))
    w_memory = np.ascontiguousarray(np.asarray(w_memory, dtype=np.float32))
    dot_scale = np.ascontiguousarray(np.asarray(dot_scale, dtype=np.float32))
    return [
        {
            "input": input[b],
            "memory": memory[b],
            "mask": mask[b],
            "w_input": w_input,
            "w_memory": w_memory,
            "dot_scale": dot_scale,
        }
        for b in range(B)
    ]


def _run_once(nc, in_maps):
    from concourse.bass_utils import run_bass_kernel_spmd

    res = run_bass_kernel_spmd(nc, in_maps, core_ids=list(range(B)))
    return np.stack([res.results[b]["out"] for b in range(B)], axis=0)


def kernel(input, memory, mask, w_input, w_memory, dot_scale):
    nc = _get_nc()
    in_maps = make_in_maps(input, memory, mask, w_input, w_memory, dot_scale)
    # The kernel is deterministic; rarely a core returns corrupted data after
    # an earlier device fault.  Run twice and require agreement.
    out = _run_once(nc, in_maps)
    for _ in range(3):
        out2 = _run_once(nc, in_maps)
        if np.array_equal(out, out2):
            return out
        out = out2
    return out
